# revision 8
# baseline (speedup 1.0000x reference)
"""Trainium2 Bass kernel: 3x3 VALID conv (NHWC) with weight thresholding + bias.

Full-input contract: kernel(x, weight, bias) -> out
  x:      (32, 56, 56, 256) fp32 NHWC
  weight: (256, 256, 3, 3)  fp32 OIHW, |w| < 0.01 -> 0
  bias:   (256,)            fp32
  out:    (32, 54, 54, 256) fp32 NHWC
Sharding: data-parallel over batch, 4 images per core on 8 cores.

Device algorithm: 1D Winograd F(2,3) along the width axis, dense (shifted
PSUM accumulation) along the height axis, implicit GEMM over channels.
Per output row pair of columns (2wt, 2wt+1):
  V0 = x[2wt]   - x[2wt+2]        (input transform, B^T, on DVE in bf16)
  V1 = x[2wt+1] + x[2wt+2]
  V2 = x[2wt+2] - x[2wt+1]
  V3 = x[2wt+1] - x[2wt+3]
  M_p[co, oh, wt] = sum_{kh, ci} U_p,kh[ci, co] V_p[ci, oh+kh, wt]   (PE)
  y[2wt]   = M0 + M1 + M2 + bias  (output transform, A^T, on DVE)
  y[2wt+1] = M1 - M2 - M3 + bias
where U = (G w) along kw: U0 = w0, U1 = (w0+w1+w2)/2, U2 = (w0-w1+w2)/2,
U3 = w2 (host-precomputed, bf16). This cuts PE column-streaming 1.5x vs
dense implicit GEMM (12 taps of N=486 per output tile vs 18).

Everything on-device is bf16 (matmul inputs, transforms); PSUM accumulates
fp32. Host converts x to bf16 (halves input DMA), de-interleaves even/odd
width columns (so all DVE reads are stride-free within rows), and converts
the bf16 output back to fp32. Error budget: ~0.3% rel, vs 2% tolerance.
"""

import numpy as np
import ml_dtypes
from contextlib import ExitStack

import concourse.bass as bass
import concourse.bacc as bacc
import concourse.tile as tile
import concourse.mybir as mybir
from concourse.bass_utils import run_bass_kernel_spmd

N_CORES = 8
IMGS_PER_CORE = 4
H, W, C = 56, 56, 256
OH, OW, CO = 54, 54, 256
P = 128
W2 = W // 2          # 28 even (or odd) columns per row
WT = OW // 2         # 27 winograd tiles per row
NPIX_IN = H * W      # 3136 = 2 * 2 * 784 (ci is separate)
ROWS_PER_BLK = 18    # 18 output rows * 27 tiles = 486 <= 512 (one PSUM bank)
N_BLKS = OH // ROWS_PER_BLK  # 3
BLK = ROWS_PER_BLK * WT      # 486
SPARSE_TH = 0.01

XCOLS_IMG = 2 * 2 * H * W2   # ci(2) x parity(2) x h(56) x w2(28) = 6272
VCOLS_CI = 4 * H * WT        # pos(4) x h(56) x wt(27) = 6048
YCOLS_IMG = N_BLKS * 2 * BLK  # blk(3) x parity(2) x 486 = 2916

TRACE = False
LAST = None
SIM_NS = None  # TimelineSim estimate of per-core exec time (filled by test.py)

_NC_CACHE = None
_last_in_maps = None

bf16 = mybir.dt.bfloat16
f32 = mybir.dt.float32


def _build_module():
    nc = bacc.Bacc(
        "TRN2",
        target_bir_lowering=False,
        debug=False,
        enable_asserts=False,
        num_devices=N_CORES,
    )
    xt = nc.dram_tensor("xt", [P, IMGS_PER_CORE * XCOLS_IMG], bf16, kind="ExternalInput").ap()
    up = nc.dram_tensor("up", [P, 48 * P], bf16, kind="ExternalInput").ap()
    b2 = nc.dram_tensor("b2", [P, 2], f32, kind="ExternalInput").ap()
    yt = nc.dram_tensor("yt", [CO, IMGS_PER_CORE * YCOLS_IMG], bf16, kind="ExternalOutput").ap()

    add = mybir.AluOpType.add
    sub = mybir.AluOpType.subtract

    with tile.TileContext(nc) as tc, ExitStack() as ctx:
        upool = ctx.enter_context(tc.tile_pool(name="u", bufs=1))
        bpool = ctx.enter_context(tc.tile_pool(name="b", bufs=1))
        xpool = ctx.enter_context(tc.tile_pool(name="x", bufs=2))
        vpool = ctx.enter_context(tc.tile_pool(name="v", bufs=2))
        tpool = ctx.enter_context(tc.tile_pool(name="t", bufs=6))
        opool = ctx.enter_context(tc.tile_pool(name="o", bufs=6))
        pspool = ctx.enter_context(tc.tile_pool(name="ps", bufs=8, space="PSUM"))

        u_sb = upool.tile([P, 48 * P], bf16)
        b_sb = bpool.tile([P, 2], f32)

        HH = H // 2          # 28 rows per h-half
        HALF = HH * W2       # 784 cols per (ci, parity, h-half)

        def load_x_half(x_tile, img, ci, par, hh):
            off = (ci * 2 + par) * (H * W2) + hh * HALF
            base = img * XCOLS_IMG + off
            nc.sync.dma_start(
                x_tile[:, off:off + HALF],
                xt[:, base:base + HALF],
            )

        # weight block index within u_sb: co-major so co=0 weights DMA first
        def tblk(co, pos, kh, ci):
            return ((co * 4 + pos) * 3 + kh) * 2 + ci

        # startup: co0 weights, then first image x (h-half0 first), then rest
        nc.sync.dma_start(u_sb[:, :24 * P], up[:, :24 * P])
        x0 = xpool.tile([P, XCOLS_IMG], bf16, tag="x", name="x_0")
        for ci in range(2):
            for par in range(2):
                load_x_half(x0, 0, ci, par, 0)
        nc.sync.dma_start(b_sb[:], b2)
        nc.sync.dma_start(u_sb[:, 24 * P:], up[:, 24 * P:])
        for ci in range(2):
            for par in range(2):
                load_x_half(x0, 0, ci, par, 1)

        for img in range(IMGS_PER_CORE):
            if img == 0:
                xc = x0
            else:
                xc = xpool.tile([P, XCOLS_IMG], bf16, tag="x", name=f"x_{img}")
                for hh in range(2):
                    for ci in range(2):
                        for par in range(2):
                            load_x_half(xc, img, ci, par, hh)

            # input transform: V[ci] tile = [pos(4) x h(56) x wt(27)]
            v = vpool.tile([P, 2 * VCOLS_CI], bf16, tag="v", name=f"v_{img}")

            def vslice(ci, pos, r0, r1):
                a = ci * VCOLS_CI + pos * (H * WT) + r0 * WT
                b = ci * VCOLS_CI + pos * (H * WT) + r1 * WT
                return v[:, a:b]

            xe = [
                xc[:, (c * 2 + 0) * H * W2:(c * 2 + 1) * H * W2].rearrange(
                    "p (h w) -> p h w", w=W2)
                for c in range(2)
            ]
            xo = [
                xc[:, (c * 2 + 1) * H * W2:(c * 2 + 2) * H * W2].rearrange(
                    "p (h w) -> p h w", w=W2)
                for c in range(2)
            ]
            # per h-half so blk0 matmuls can start after half 0 lands
            for hh in range(2):
                r0, r1 = hh * HH, (hh + 1) * HH
                for c in range(2):
                    e0 = xe[c][:, r0:r1, 0:WT]
                    e1 = xe[c][:, r0:r1, 1:WT + 1]
                    o0 = xo[c][:, r0:r1, 0:WT]
                    o1 = xo[c][:, r0:r1, 1:WT + 1]
                    vw = [
                        vslice(c, pos, r0, r1).rearrange("p (h w) -> p h w", w=WT)
                        for pos in range(4)
                    ]
                    nc.vector.tensor_tensor(vw[0], e0, e1, sub)
                    nc.vector.tensor_tensor(vw[1], o0, e1, add)
                    nc.vector.tensor_tensor(vw[2], e1, o0, sub)
                    nc.vector.tensor_tensor(vw[3], o0, o1, sub)

            for blk in range(N_BLKS):
                oh0 = blk * ROWS_PER_BLK
                for co in range(2):
                    ps = [
                        pspool.tile([P, BLK], f32, tag="ps",
                                    name=f"ps_{img}_{blk}_{co}_{pos}")
                        for pos in range(4)
                    ]
                    for pos in range(4):
                        mm = 0
                        for kh in range(3):
                            for ci in range(2):
                                t = tblk(co, pos, kh, ci)
                                rhs = vslice(ci, pos, oh0 + kh, oh0 + kh + ROWS_PER_BLK)
                                nc.tensor.matmul(
                                    ps[pos][:],
                                    u_sb[:, t * P:(t + 1) * P],
                                    rhs,
                                    start=(mm == 0),
                                    stop=(mm == 5),
                                )
                                mm += 1
                    # output transform + bias (A^T). DVE ops may read at most
                    # one PSUM operand, so ACT computes a = m1 + bias first.
                    bias = b_sb[:, co:co + 1]
                    mult = mybir.AluOpType.mult
                    a_t = tpool.tile([P, BLK], f32, tag="t", name=f"a_{img}_{blk}_{co}")
                    t0 = tpool.tile([P, BLK], f32, tag="t", name=f"t0_{img}_{blk}_{co}")
                    t1 = tpool.tile([P, BLK], f32, tag="t", name=f"t1_{img}_{blk}_{co}")
                    yo = opool.tile([P, 2 * BLK], bf16, tag="yo", name=f"y_{img}_{blk}_{co}")
                    nc.scalar.activation(a_t[:], ps[1][:],
                                         mybir.ActivationFunctionType.Identity,
                                         bias=bias, scale=1.0)      # a = m1 + bias
                    nc.vector.tensor_tensor(t0[:], ps[0][:], a_t[:], add)   # m0 + a
                    nc.vector.tensor_tensor(yo[:, :BLK], ps[2][:], t0[:], add)  # y_e = m2 + t0
                    nc.vector.scalar_tensor_tensor(t1[:], ps[2][:], -1.0, a_t[:], mult, add)  # a - m2
                    nc.vector.scalar_tensor_tensor(yo[:, BLK:], ps[3][:], -1.0, t1[:], mult, add)  # t1 - m3
                    col0 = img * YCOLS_IMG + blk * 2 * BLK
                    nc.sync.dma_start(yt[co * P:(co + 1) * P, col0:col0 + 2 * BLK], yo[:])
    nc.compile()
    return nc


def _marshal(x, weight, bias):
    """Host-side sharding + layout. Returns per-core input maps."""
    x = np.ascontiguousarray(np.asarray(x, dtype=np.float32))
    weight = np.asarray(weight, dtype=np.float32)
    bias = np.asarray(bias, dtype=np.float32)

    # weights: threshold, then 1D Winograd G-transform along kw, pack bf16
    w = np.where(np.abs(weight) < SPARSE_TH, np.float32(0.0), weight)
    wt = w.transpose(1, 2, 3, 0)                # [ci, kh, kw, co]
    u = np.empty((4, 256, 3, 256), np.float32)   # [pos, ci, kh, co]
    u[0] = wt[:, :, 0]
    u[1] = (wt[:, :, 0] + wt[:, :, 1] + wt[:, :, 2]) * 0.5
    u[2] = (wt[:, :, 0] - wt[:, :, 1] + wt[:, :, 2]) * 0.5
    u[3] = wt[:, :, 2]
    # -> [ci_in(128), co(2), pos(4), kh(3), ci_chunk(2), co_in(128)]
    u6 = u.reshape(4, 2, P, 3, 2, P)            # [pos, ci_c, ci_in, kh, co_c, co_in]
    up = np.ascontiguousarray(
        u6.transpose(2, 4, 0, 3, 1, 5).reshape(P, 48 * P)
    ).astype(ml_dtypes.bfloat16)
    b2 = np.ascontiguousarray(bias.reshape(2, P).T)   # [co_in, co_chunk]

    xb = x.astype(ml_dtypes.bfloat16)
    in_maps = []
    for i in range(N_CORES):
        xc = xb[i * IMGS_PER_CORE:(i + 1) * IMGS_PER_CORE]   # [4,56,56,256]
        # -> [ci_in(128), img, ci_chunk, parity, h, w2]
        x6 = xc.reshape(IMGS_PER_CORE, H, W2, 2, 2, P)  # [img,h,w2,parity,ci_c,ci_in]
        xt_i = np.ascontiguousarray(
            x6.transpose(5, 0, 4, 3, 1, 2).reshape(P, IMGS_PER_CORE * XCOLS_IMG)
        )
        in_maps.append({"xt": xt_i, "up": up, "b2": b2})
    return in_maps


def kernel(x, weight, bias):
    global _NC_CACHE, LAST, _last_in_maps
    in_maps = _marshal(x, weight, bias)

    if _NC_CACHE is None:
        _NC_CACHE = _build_module()
    nc = _NC_CACHE
    _last_in_maps = in_maps

    LAST = run_bass_kernel_spmd(
        nc, in_maps, core_ids=list(range(N_CORES)), trace=TRACE
    )

    out = np.empty((32, OH, OW, CO), np.float32)
    for i in range(N_CORES):
        ytc = np.asarray(LAST.results[i]["yt"]).astype(np.float32)  # [256, 4*2916]
        # cols: [img, blk, parity, r(18), wt(27)]
        y6 = ytc.reshape(2, P, IMGS_PER_CORE, N_BLKS, 2, ROWS_PER_BLK, WT)
        # -> [img, oh(blk,r), ow(wt,parity), co(chunk,co_in)]
        out[i * IMGS_PER_CORE:(i + 1) * IMGS_PER_CORE] = (
            y6.transpose(2, 3, 5, 6, 4, 0, 1)
            .reshape(IMGS_PER_CORE, OH, OW, CO)
        )
    return out


# revision 12
# speedup vs baseline: 1.0031x; 1.0031x over previous
"""Trainium2 Bass kernel: 3x3 VALID conv (NHWC) with weight thresholding + bias.

Full-input contract: kernel(x, weight, bias) -> out
  x:      (32, 56, 56, 256) fp32 NHWC
  weight: (256, 256, 3, 3)  fp32 OIHW, |w| < 0.01 -> 0
  bias:   (256,)            fp32
  out:    (32, 54, 54, 256) fp32 NHWC
Sharding: data-parallel over batch, 4 images per core on 8 cores.

Device algorithm: 1D Winograd F(2,3) along the width axis, dense (shifted
PSUM accumulation) along the height axis, implicit GEMM over channels.
Per output row pair of columns (2wt, 2wt+1):
  V0 = x[2wt]   - x[2wt+2]        (input transform, B^T, on DVE in bf16)
  V1 = x[2wt+1] + x[2wt+2]
  V2 = x[2wt+2] - x[2wt+1]
  V3 = x[2wt+1] - x[2wt+3]
  M_p[co, oh, wt] = sum_{kh, ci} U_p,kh[ci, co] V_p[ci, oh+kh, wt]   (PE)
  y[2wt]   = M0 + M1 + M2 + bias  (output transform, A^T, on DVE)
  y[2wt+1] = M1 - M2 - M3 + bias
where U = (G w) along kw: U0 = w0, U1 = (w0+w1+w2)/2, U2 = (w0-w1+w2)/2,
U3 = w2 (host-precomputed, bf16). This cuts PE column-streaming 1.5x vs
dense implicit GEMM (12 taps of N=486 per output tile vs 18).

Everything on-device is bf16 (matmul inputs, transforms); PSUM accumulates
fp32. Host converts x to bf16 (halves input DMA), de-interleaves even/odd
width columns (so all DVE reads are stride-free within rows), and converts
the bf16 output back to fp32. Error budget: ~0.3% rel, vs 2% tolerance.
"""

import numpy as np
import ml_dtypes
from contextlib import ExitStack

import concourse.bass as bass
import concourse.bacc as bacc
import concourse.tile as tile
import concourse.mybir as mybir
from concourse.bass_utils import run_bass_kernel_spmd

N_CORES = 8
IMGS_PER_CORE = 4
H, W, C = 56, 56, 256
OH, OW, CO = 54, 54, 256
P = 128
W2 = W // 2          # 28 even (or odd) columns per row
WT = OW // 2         # 27 winograd tiles per row
NPIX_IN = H * W      # 3136 = 2 * 2 * 784 (ci is separate)
ROWS_PER_BLK = 18    # 18 output rows * 27 tiles = 486 <= 512 (one PSUM bank)
N_BLKS = OH // ROWS_PER_BLK  # 3
BLK = ROWS_PER_BLK * WT      # 486
SPARSE_TH = 0.01

XCOLS_IMG = 2 * 2 * H * W2   # ci(2) x parity(2) x h(56) x w2(28) = 6272
VCOLS_CI = 4 * H * WT        # pos(4) x h(56) x wt(27) = 6048
YCOLS_IMG = N_BLKS * 2 * BLK  # blk(3) x parity(2) x 486 = 2916

TRACE = False
LAST = None
SIM_NS = None  # TimelineSim estimate of per-core exec time (filled by test.py)

_NC_CACHE = None
_last_in_maps = None

bf16 = mybir.dt.bfloat16
f32 = mybir.dt.float32


def _build_module():
    nc = bacc.Bacc(
        "TRN2",
        target_bir_lowering=False,
        debug=False,
        enable_asserts=False,
        num_devices=N_CORES,
    )
    xt = nc.dram_tensor("xt", [P, IMGS_PER_CORE * XCOLS_IMG], bf16, kind="ExternalInput").ap()
    up = nc.dram_tensor("up", [P, 48 * P], bf16, kind="ExternalInput").ap()
    b2 = nc.dram_tensor("b2", [P, 2], f32, kind="ExternalInput").ap()
    yt = nc.dram_tensor("yt", [CO, IMGS_PER_CORE * YCOLS_IMG], bf16, kind="ExternalOutput").ap()

    add = mybir.AluOpType.add
    sub = mybir.AluOpType.subtract

    with tile.TileContext(nc) as tc, ExitStack() as ctx:
        upool = ctx.enter_context(tc.tile_pool(name="u", bufs=1))
        bpool = ctx.enter_context(tc.tile_pool(name="b", bufs=1))
        xpool = ctx.enter_context(tc.tile_pool(name="x", bufs=2))
        vpool = ctx.enter_context(tc.tile_pool(name="v", bufs=2))
        tpool = ctx.enter_context(tc.tile_pool(name="t", bufs=6))
        opool = ctx.enter_context(tc.tile_pool(name="o", bufs=6))
        pspool = ctx.enter_context(tc.tile_pool(name="ps", bufs=8, space="PSUM"))

        u_sb = upool.tile([P, 48 * P], bf16)
        b_sb = bpool.tile([P, 2], f32)

        HH = H // 2          # 28 rows per h-half
        HALF = HH * W2       # 784 cols per (ci, parity, h-half)
        CICOLS = H * W2      # 1568 cols per (ci, parity)

        def load_x_block(x_tile, img, a, b):
            """One DMA covering cols [a, b) of BOTH ci chunks (3D strided AP)."""
            src = xt[:, img * XCOLS_IMG:(img + 1) * XCOLS_IMG].rearrange(
                "p (ci r) -> p ci r", ci=2)[:, :, a:b]
            dst = x_tile[:].rearrange("p (ci r) -> p ci r", ci=2)[:, :, a:b]
            nc.sync.dma_start(dst, src)

        # weight block index within u_sb: co-major so co=0 weights DMA first
        def tblk(co, pos, kh, ci):
            return ((co * 4 + pos) * 3 + kh) * 2 + ci

        # Startup order. The HWDGE dispatches DMAs serially (~625ns each), so
        # order by first-consumer: V pos0 only reads even-parity columns, so
        # x par0-h0 goes first, then the pos0 weight block, then par1.
        def load_u(pos0, pos1):
            c0 = tblk(0, pos0, 0, 0) * P
            c1 = (tblk(0, pos1, 2, 1) + 1) * P
            nc.scalar.dma_start(u_sb[:, c0:c1], up[:, c0:c1])

        x0 = xpool.tile([P, XCOLS_IMG], bf16, tag="x", name="x_0")
        load_x_block(x0, 0, 0, HALF)               # par0, h-half0 (V pos0)
        load_u(0, 0)                               # weights co0/pos0
        load_x_block(x0, 0, CICOLS, CICOLS + HALF)  # par1, h-half0
        load_u(1, 1)
        nc.sync.dma_start(b_sb[:], b2)
        load_x_block(x0, 0, HALF, CICOLS)          # par0, h-half1
        load_x_block(x0, 0, CICOLS + HALF, 2 * CICOLS)  # par1, h-half1
        load_u(2, 3)
        nc.scalar.dma_start(u_sb[:, 24 * P:], up[:, 24 * P:])  # co1

        for img in range(IMGS_PER_CORE):
            if img == 0:
                xc = x0
            else:
                xc = xpool.tile([P, XCOLS_IMG], bf16, tag="x", name=f"x_{img}")
                for par in range(2):
                    load_x_block(xc, img, par * CICOLS, (par + 1) * CICOLS)

            # input transform: V[ci] tile = [pos(4) x h(56) x wt(27)]
            v = vpool.tile([P, 2 * VCOLS_CI], bf16, tag="v", name=f"v_{img}")

            def vslice(ci, pos, r0, r1):
                a = ci * VCOLS_CI + pos * (H * WT) + r0 * WT
                b = ci * VCOLS_CI + pos * (H * WT) + r1 * WT
                return v[:, a:b]

            xe = [
                xc[:, (c * 2 + 0) * H * W2:(c * 2 + 1) * H * W2].rearrange(
                    "p (h w) -> p h w", w=W2)
                for c in range(2)
            ]
            xo = [
                xc[:, (c * 2 + 1) * H * W2:(c * 2 + 2) * H * W2].rearrange(
                    "p (h w) -> p h w", w=W2)
                for c in range(2)
            ]
            # per h-half so blk0 matmuls can start after half 0 lands;
            # pos-major, ci interleaved: first unit's pos0 deps resolve first
            for hh in range(2):
                r0, r1 = hh * HH, (hh + 1) * HH
                for pos in range(4):
                    for c in range(2):
                        e0 = xe[c][:, r0:r1, 0:WT]
                        e1 = xe[c][:, r0:r1, 1:WT + 1]
                        o0 = xo[c][:, r0:r1, 0:WT]
                        o1 = xo[c][:, r0:r1, 1:WT + 1]
                        vw = vslice(c, pos, r0, r1).rearrange("p (h w) -> p h w", w=WT)
                        if pos == 0:
                            nc.vector.tensor_tensor(vw, e0, e1, sub)
                        elif pos == 1:
                            nc.vector.tensor_tensor(vw, o0, e1, add)
                        elif pos == 2:
                            nc.vector.tensor_tensor(vw, e1, o0, sub)
                        else:
                            nc.vector.tensor_tensor(vw, o0, o1, sub)

            for blk in range(N_BLKS):
                oh0 = blk * ROWS_PER_BLK
                for co in range(2):
                    ps = [
                        pspool.tile([P, BLK], f32, tag="ps",
                                    name=f"ps_{img}_{blk}_{co}_{pos}")
                        for pos in range(4)
                    ]
                    for pos in range(4):
                        mm = 0
                        for kh in range(3):
                            for ci in range(2):
                                t = tblk(co, pos, kh, ci)
                                rhs = vslice(ci, pos, oh0 + kh, oh0 + kh + ROWS_PER_BLK)
                                nc.tensor.matmul(
                                    ps[pos][:],
                                    u_sb[:, t * P:(t + 1) * P],
                                    rhs,
                                    start=(mm == 0),
                                    stop=(mm == 5),
                                )
                                mm += 1
                    # output transform + bias (A^T). DVE ops may read at most
                    # one PSUM operand, so ACT computes a = m1 + bias first.
                    bias = b_sb[:, co:co + 1]
                    mult = mybir.AluOpType.mult
                    a_t = tpool.tile([P, BLK], f32, tag="t", name=f"a_{img}_{blk}_{co}")
                    t0 = tpool.tile([P, BLK], f32, tag="t", name=f"t0_{img}_{blk}_{co}")
                    t1 = tpool.tile([P, BLK], f32, tag="t", name=f"t1_{img}_{blk}_{co}")
                    yo = opool.tile([P, 2 * BLK], bf16, tag="yo", name=f"y_{img}_{blk}_{co}")
                    nc.scalar.activation(a_t[:], ps[1][:],
                                         mybir.ActivationFunctionType.Identity,
                                         bias=bias, scale=1.0)      # a = m1 + bias
                    nc.vector.tensor_tensor(t0[:], ps[0][:], a_t[:], add)   # m0 + a
                    nc.vector.scalar_tensor_tensor(t1[:], ps[2][:], -1.0, a_t[:], mult, add)  # a - m2
                    nc.vector.tensor_tensor(yo[:, :BLK], ps[2][:], t0[:], add)  # y_e = m2 + t0
                    nc.vector.scalar_tensor_tensor(yo[:, BLK:], ps[3][:], -1.0, t1[:], mult, add)  # t1 - m3
                    col0 = img * YCOLS_IMG + blk * 2 * BLK
                    nc.scalar.dma_start(yt[co * P:(co + 1) * P, col0:col0 + BLK], yo[:, :BLK])
                    nc.scalar.dma_start(yt[co * P:(co + 1) * P, col0 + BLK:col0 + 2 * BLK], yo[:, BLK:])
    nc.compile()
    return nc


def _marshal(x, weight, bias):
    """Host-side sharding + layout. Returns per-core input maps."""
    x = np.ascontiguousarray(np.asarray(x, dtype=np.float32))
    weight = np.asarray(weight, dtype=np.float32)
    bias = np.asarray(bias, dtype=np.float32)

    # weights: threshold, then 1D Winograd G-transform along kw, pack bf16
    w = np.where(np.abs(weight) < SPARSE_TH, np.float32(0.0), weight)
    wt = w.transpose(1, 2, 3, 0)                # [ci, kh, kw, co]
    u = np.empty((4, 256, 3, 256), np.float32)   # [pos, ci, kh, co]
    u[0] = wt[:, :, 0]
    u[1] = (wt[:, :, 0] + wt[:, :, 1] + wt[:, :, 2]) * 0.5
    u[2] = (wt[:, :, 0] - wt[:, :, 1] + wt[:, :, 2]) * 0.5
    u[3] = wt[:, :, 2]
    # -> [ci_in(128), co(2), pos(4), kh(3), ci_chunk(2), co_in(128)]
    u6 = u.reshape(4, 2, P, 3, 2, P)            # [pos, ci_c, ci_in, kh, co_c, co_in]
    up = np.ascontiguousarray(
        u6.transpose(2, 4, 0, 3, 1, 5).reshape(P, 48 * P)
    ).astype(ml_dtypes.bfloat16)
    b2 = np.ascontiguousarray(bias.reshape(2, P).T)   # [co_in, co_chunk]

    xb = x.astype(ml_dtypes.bfloat16)
    in_maps = []
    for i in range(N_CORES):
        xc = xb[i * IMGS_PER_CORE:(i + 1) * IMGS_PER_CORE]   # [4,56,56,256]
        # -> [ci_in(128), img, ci_chunk, parity, h, w2]
        x6 = xc.reshape(IMGS_PER_CORE, H, W2, 2, 2, P)  # [img,h,w2,parity,ci_c,ci_in]
        xt_i = np.ascontiguousarray(
            x6.transpose(5, 0, 4, 3, 1, 2).reshape(P, IMGS_PER_CORE * XCOLS_IMG)
        )
        in_maps.append({"xt": xt_i, "up": up, "b2": b2})
    return in_maps


def kernel(x, weight, bias):
    global _NC_CACHE, LAST, _last_in_maps
    in_maps = _marshal(x, weight, bias)

    if _NC_CACHE is None:
        _NC_CACHE = _build_module()
    nc = _NC_CACHE
    _last_in_maps = in_maps

    LAST = run_bass_kernel_spmd(
        nc, in_maps, core_ids=list(range(N_CORES)), trace=TRACE
    )

    out = np.empty((32, OH, OW, CO), np.float32)
    for i in range(N_CORES):
        ytc = np.asarray(LAST.results[i]["yt"]).astype(np.float32)  # [256, 4*2916]
        # cols: [img, blk, parity, r(18), wt(27)]
        y6 = ytc.reshape(2, P, IMGS_PER_CORE, N_BLKS, 2, ROWS_PER_BLK, WT)
        # -> [img, oh(blk,r), ow(wt,parity), co(chunk,co_in)]
        out[i * IMGS_PER_CORE:(i + 1) * IMGS_PER_CORE] = (
            y6.transpose(2, 3, 5, 6, 4, 0, 1)
            .reshape(IMGS_PER_CORE, OH, OW, CO)
        )
    return out


# revision 21
# speedup vs baseline: 1.0246x; 1.0215x over previous
"""Trainium2 Bass kernel: 3x3 VALID conv (NHWC) with weight thresholding + bias.

Full-input contract: kernel(x, weight, bias) -> out
  x:      (32, 56, 56, 256) fp32 NHWC
  weight: (256, 256, 3, 3)  fp32 OIHW, |w| < 0.01 -> 0
  bias:   (256,)            fp32
  out:    (32, 54, 54, 256) fp32 NHWC
Sharding: data-parallel over batch, 4 images per core on 8 cores.

Device algorithm: 1D Winograd F(2,3) along the width axis, dense (shifted
PSUM accumulation) along the height axis, implicit GEMM over channels.
Per output row pair of columns (2wt, 2wt+1):
  V0 = x[2wt]   - x[2wt+2]        (input transform, B^T, on DVE in bf16)
  V1 = x[2wt+1] + x[2wt+2]
  V2 = x[2wt+2] - x[2wt+1]
  V3 = x[2wt+1] - x[2wt+3]
  M_p[co, oh, wt] = sum_{kh, ci} U_p,kh[ci, co] V_p[ci, oh+kh, wt]   (PE)
  y[2wt]   = M0 + M1 + M2 + bias  (output transform, A^T, on DVE)
  y[2wt+1] = M1 - M2 - M3 + bias
where U = (G w) along kw: U0 = w0, U1 = (w0+w1+w2)/2, U2 = (w0-w1+w2)/2,
U3 = w2 (host-precomputed, bf16). This cuts PE column-streaming 1.5x vs
dense implicit GEMM (12 taps of N=486 per output tile vs 18).

Everything on-device is bf16 (matmul inputs, transforms); PSUM accumulates
fp32. Host converts x to bf16 (halves input DMA), de-interleaves even/odd
width columns (so all DVE reads are stride-free within rows), and converts
the bf16 output back to fp32. Error budget: ~0.3% rel, vs 2% tolerance.
"""

import numpy as np
import ml_dtypes
from contextlib import ExitStack

import concourse.bass as bass
import concourse.bacc as bacc
import concourse.tile as tile
import concourse.mybir as mybir
from concourse.bass_utils import run_bass_kernel_spmd

N_CORES = 8
IMGS_PER_CORE = 4
H, W, C = 56, 56, 256
OH, OW, CO = 54, 54, 256
P = 128
W2 = W // 2          # 28 even (or odd) columns per row
WT = OW // 2         # 27 winograd tiles per row
NPIX_IN = H * W      # 3136 = 2 * 2 * 784 (ci is separate)
ROWS_PER_BLK = 18    # 18 output rows * 27 tiles = 486 <= 512 (one PSUM bank)
N_BLKS = OH // ROWS_PER_BLK  # 3
BLK = ROWS_PER_BLK * WT      # 486
SPARSE_TH = 0.01

XCOLS_IMG = 2 * 2 * H * W2   # ci(2) x parity(2) x h(56) x w2(28) = 6272
VCOLS_CI = 4 * H * WT        # pos(4) x h(56) x wt(27) = 6048
YCOLS_IMG = N_BLKS * 2 * BLK  # blk(3) x parity(2) x 486 = 2916

TRACE = False
LAST = None
SIM_NS = None  # TimelineSim estimate of per-core exec time (filled by test.py)

_NC_CACHE = None
_last_in_maps = None

bf16 = mybir.dt.bfloat16
f32 = mybir.dt.float32


def _build_module():
    nc = bacc.Bacc(
        "TRN2",
        target_bir_lowering=False,
        debug=False,
        enable_asserts=False,
        num_devices=N_CORES,
    )
    xt = nc.dram_tensor("xt", [P, IMGS_PER_CORE * XCOLS_IMG], bf16, kind="ExternalInput").ap()
    up = nc.dram_tensor("up", [P, 48 * P], bf16, kind="ExternalInput").ap()
    b2 = nc.dram_tensor("b2", [P, 2], f32, kind="ExternalInput").ap()
    yt = nc.dram_tensor("yt", [CO, IMGS_PER_CORE * YCOLS_IMG], bf16, kind="ExternalOutput").ap()

    add = mybir.AluOpType.add
    sub = mybir.AluOpType.subtract

    with tile.TileContext(nc) as tc, ExitStack() as ctx:
        upool = ctx.enter_context(tc.tile_pool(name="u", bufs=1))
        bpool = ctx.enter_context(tc.tile_pool(name="b", bufs=1))
        xpool = ctx.enter_context(tc.tile_pool(name="x", bufs=2))
        vpool = ctx.enter_context(tc.tile_pool(name="v", bufs=2))
        tpool = ctx.enter_context(tc.tile_pool(name="t", bufs=6))
        opool = ctx.enter_context(tc.tile_pool(name="o", bufs=6))
        pspool = ctx.enter_context(tc.tile_pool(name="ps", bufs=8, space="PSUM"))

        u_sb = upool.tile([P, 48 * P], bf16)
        b_sb = bpool.tile([P, 2], f32)

        # PE ramp warmup: the tensor engine runs at half clock until it has
        # been continuously busy ~3us (HAM). Real matmuls can't start until
        # x + weights land (~4us), so fill the wait with tiny dummy matmuls
        # and let the real stream begin already warm.
        NWARM = 80
        w_warm = upool.tile([P, 64], bf16)
        nc.gpsimd.memset(w_warm[:], 0.0)
        ps_warm = pspool.tile([P, BLK], f32, tag="ps", name="ps_warm")
        for i in range(NWARM):
            nc.tensor.matmul(ps_warm[:64, :64], w_warm[:], w_warm[:],
                             start=(i == 0), stop=(i == NWARM - 1))

        HH = H // 2          # 28 rows per h-half
        HALF = HH * W2       # 784 cols per (ci, parity, h-half)
        CICOLS = H * W2      # 1568 cols per (ci, parity)

        def load_x_block(x_tile, img, a, b):
            """One DMA covering cols [a, b) of BOTH ci chunks (3D strided AP)."""
            src = xt[:, img * XCOLS_IMG:(img + 1) * XCOLS_IMG].rearrange(
                "p (ci r) -> p ci r", ci=2)[:, :, a:b]
            dst = x_tile[:].rearrange("p (ci r) -> p ci r", ci=2)[:, :, a:b]
            nc.sync.dma_start(dst, src)

        def load_x_ci(x_tile, img, ci, a, b):
            base = img * XCOLS_IMG + ci * (2 * CICOLS)
            off = ci * (2 * CICOLS)
            nc.sync.dma_start(x_tile[:, off + a:off + b],
                              xt[:, base + a:base + b])

        # weight block index within u_sb: co-major so co=0 weights DMA first
        def tblk(co, pos, kh, ci):
            return ((co * 4 + pos) * 3 + kh) * 2 + ci

        # Startup order. The HWDGE dispatches DMAs serially (~625ns each), so
        # order by first-consumer: V pos0 only reads even-parity columns, so
        # x par0-h0 goes first, then the pos0 weight block, then par1.
        def load_u(pos0, pos1):
            c0 = tblk(0, pos0, 0, 0) * P
            c1 = (tblk(0, pos1, 2, 1) + 1) * P
            nc.scalar.dma_start(u_sb[:, c0:c1], up[:, c0:c1])

        x0 = xpool.tile([P, XCOLS_IMG], bf16, tag="x", name="x_0")
        load_x_block(x0, 0, 0, HALF)               # par0, h-half0 (V pos0)
        load_u(0, 0)                               # weights co0/pos0
        load_x_block(x0, 0, CICOLS, CICOLS + HALF)  # par1, h-half0
        load_u(1, 1)
        nc.sync.dma_start(b_sb[:], b2)
        load_x_block(x0, 0, HALF, CICOLS)          # par0, h-half1
        load_x_block(x0, 0, CICOLS + HALF, 2 * CICOLS)  # par1, h-half1
        load_u(2, 3)
        nc.scalar.dma_start(u_sb[:, 24 * P:], up[:, 24 * P:])  # co1

        for img in range(IMGS_PER_CORE):
            if img == 0:
                xc = x0
            else:
                xc = xpool.tile([P, XCOLS_IMG], bf16, tag="x", name=f"x_{img}")
                for par in range(2):
                    load_x_block(xc, img, par * CICOLS, (par + 1) * CICOLS)

            # input transform: V[ci] tile = [pos(4) x h(56) x wt(27)]
            v = vpool.tile([P, 2 * VCOLS_CI], bf16, tag="v", name=f"v_{img}")

            def vslice(ci, pos, r0, r1):
                a = ci * VCOLS_CI + pos * (H * WT) + r0 * WT
                b = ci * VCOLS_CI + pos * (H * WT) + r1 * WT
                return v[:, a:b]

            xe = [
                xc[:, (c * 2 + 0) * H * W2:(c * 2 + 1) * H * W2].rearrange(
                    "p (h w) -> p h w", w=W2)
                for c in range(2)
            ]
            xo = [
                xc[:, (c * 2 + 1) * H * W2:(c * 2 + 2) * H * W2].rearrange(
                    "p (h w) -> p h w", w=W2)
                for c in range(2)
            ]
            # per h-half so blk0 matmuls can start after half 0 lands;
            # pos-major, ci interleaved: first unit's pos0 deps resolve first
            for hh in range(2):
                r0, r1 = hh * HH, (hh + 1) * HH
                for pos in range(4):
                    for c in range(2):
                        e0 = xe[c][:, r0:r1, 0:WT]
                        e1 = xe[c][:, r0:r1, 1:WT + 1]
                        o0 = xo[c][:, r0:r1, 0:WT]
                        o1 = xo[c][:, r0:r1, 1:WT + 1]
                        vw = vslice(c, pos, r0, r1).rearrange("p (h w) -> p h w", w=WT)
                        if pos == 0:
                            nc.vector.tensor_tensor(vw, e0, e1, sub)
                        elif pos == 1:
                            nc.vector.tensor_tensor(vw, o0, e1, add)
                        elif pos == 2:
                            nc.vector.tensor_tensor(vw, e1, o0, sub)
                        else:
                            nc.vector.tensor_tensor(vw, o0, o1, sub)

            def unit(img, blk, co, c0, c1, tag2):
                """Matmuls + output transform for output cols [c0, c1) of the
                (blk, co) tile. DVE ops may read at most one PSUM operand, so
                ACT computes a = m1 + bias first."""
                n = c1 - c0
                oh0 = blk * ROWS_PER_BLK
                ps = [
                    pspool.tile([P, n], f32, tag="ps",
                                name=f"ps_{img}_{blk}_{co}_{pos}{tag2}")
                    for pos in range(4)
                ]
                for pos in range(4):
                    mm = 0
                    for kh in range(3):
                        for ci in range(2):
                            t = tblk(co, pos, kh, ci)
                            rhs = vslice(ci, pos, oh0 + kh, oh0 + kh + ROWS_PER_BLK)[:, c0:c1]
                            nc.tensor.matmul(
                                ps[pos][:],
                                u_sb[:, t * P:(t + 1) * P],
                                rhs,
                                start=(mm == 0),
                                stop=(mm == 5),
                            )
                            mm += 1
                bias = b_sb[:, co:co + 1]
                mult = mybir.AluOpType.mult
                a_t = tpool.tile([P, n], f32, tag="t", name=f"a_{img}_{blk}_{co}{tag2}")
                t0 = tpool.tile([P, n], f32, tag="t", name=f"t0_{img}_{blk}_{co}{tag2}")
                t1 = tpool.tile([P, n], f32, tag="t", name=f"t1_{img}_{blk}_{co}{tag2}")
                yo = opool.tile([P, 2 * n], bf16, tag="yo", name=f"y_{img}_{blk}_{co}{tag2}")
                nc.scalar.activation(a_t[:], ps[1][:],
                                     mybir.ActivationFunctionType.Identity,
                                     bias=bias, scale=1.0)      # a = m1 + bias
                nc.vector.tensor_tensor(t0[:], ps[0][:], a_t[:], add)   # m0 + a
                nc.vector.scalar_tensor_tensor(t1[:], ps[2][:], -1.0, a_t[:], mult, add)  # a - m2
                nc.vector.tensor_tensor(yo[:, :n], ps[2][:], t0[:], add)  # y_e = m2 + t0
                nc.vector.scalar_tensor_tensor(yo[:, n:], ps[3][:], -1.0, t1[:], mult, add)  # t1 - m3
                col0 = img * YCOLS_IMG + blk * 2 * BLK
                nc.scalar.dma_start(yt[co * P:(co + 1) * P, col0 + c0:col0 + c1], yo[:, :n])
                nc.scalar.dma_start(yt[co * P:(co + 1) * P, col0 + BLK + c0:col0 + BLK + c1], yo[:, n:])

            for blk in range(N_BLKS):
                for co in range(2):
                    unit(img, blk, co, 0, BLK, "")
    nc.compile()
    return nc


def _marshal(x, weight, bias):
    """Host-side sharding + layout. Returns per-core input maps."""
    x = np.ascontiguousarray(np.asarray(x, dtype=np.float32))
    weight = np.asarray(weight, dtype=np.float32)
    bias = np.asarray(bias, dtype=np.float32)

    # weights: threshold, then 1D Winograd G-transform along kw, pack bf16
    w = np.where(np.abs(weight) < SPARSE_TH, np.float32(0.0), weight)
    wt = w.transpose(1, 2, 3, 0)                # [ci, kh, kw, co]
    u = np.empty((4, 256, 3, 256), np.float32)   # [pos, ci, kh, co]
    u[0] = wt[:, :, 0]
    u[1] = (wt[:, :, 0] + wt[:, :, 1] + wt[:, :, 2]) * 0.5
    u[2] = (wt[:, :, 0] - wt[:, :, 1] + wt[:, :, 2]) * 0.5
    u[3] = wt[:, :, 2]
    # -> [ci_in(128), co(2), pos(4), kh(3), ci_chunk(2), co_in(128)]
    u6 = u.reshape(4, 2, P, 3, 2, P)            # [pos, ci_c, ci_in, kh, co_c, co_in]
    up = np.ascontiguousarray(
        u6.transpose(2, 4, 0, 3, 1, 5).reshape(P, 48 * P)
    ).astype(ml_dtypes.bfloat16)
    b2 = np.ascontiguousarray(bias.reshape(2, P).T)   # [co_in, co_chunk]

    xb = x.astype(ml_dtypes.bfloat16)
    in_maps = []
    for i in range(N_CORES):
        xc = xb[i * IMGS_PER_CORE:(i + 1) * IMGS_PER_CORE]   # [4,56,56,256]
        # -> [ci_in(128), img, ci_chunk, parity, h, w2]
        x6 = xc.reshape(IMGS_PER_CORE, H, W2, 2, 2, P)  # [img,h,w2,parity,ci_c,ci_in]
        xt_i = np.ascontiguousarray(
            x6.transpose(5, 0, 4, 3, 1, 2).reshape(P, IMGS_PER_CORE * XCOLS_IMG)
        )
        in_maps.append({"xt": xt_i, "up": up, "b2": b2})
    return in_maps


def kernel(x, weight, bias):
    global _NC_CACHE, LAST, _last_in_maps
    in_maps = _marshal(x, weight, bias)

    if _NC_CACHE is None:
        _NC_CACHE = _build_module()
    nc = _NC_CACHE
    _last_in_maps = in_maps

    LAST = run_bass_kernel_spmd(
        nc, in_maps, core_ids=list(range(N_CORES)), trace=TRACE
    )

    out = np.empty((32, OH, OW, CO), np.float32)
    for i in range(N_CORES):
        ytc = np.asarray(LAST.results[i]["yt"]).astype(np.float32)  # [256, 4*2916]
        # cols: [img, blk, parity, r(18), wt(27)]
        y6 = ytc.reshape(2, P, IMGS_PER_CORE, N_BLKS, 2, ROWS_PER_BLK, WT)
        # -> [img, oh(blk,r), ow(wt,parity), co(chunk,co_in)]
        out[i * IMGS_PER_CORE:(i + 1) * IMGS_PER_CORE] = (
            y6.transpose(2, 3, 5, 6, 4, 0, 1)
            .reshape(IMGS_PER_CORE, OH, OW, CO)
        )
    return out
